# revision 9
# baseline (speedup 1.0000x reference)
"""Trainium2 Bass kernel for the DCN cross layer.

Computes out = x0 * (x_cross @ w)[:, None] + b + x_cross for
x0, x_cross: [16384, 4096] f32, w, b: [4096] f32.

Sharding: pure data parallel — batch split across 8 NeuronCores,
w and b replicated. Each core processes a [2048, 4096] shard.

The kernel is HBM-bandwidth bound (~358 GB/s per core). Inputs are
downcast to fp16 on the host during sharding and the output is stored
as fp16 and upcast on the host, halving HBM traffic (96 MB -> 48 MB
per core). The x_cross @ w row-dot accumulates into an f32 tile so
precision loss is only the fp16 input rounding (~4e-4 rel norm vs the
2e-2 gate).

Engine split (fp16 DVE 2x packing does not engage for
scalar_tensor_tensor, so a 3-pass all-DVE loop is vector-bound):
  DVE : s = rowsum(xc * w)            (scalar_tensor_tensor accum, 1x)
  ACT : t = x0 * s + b0               (per-partition scale=s, bias=b0)
  DVE : out = t + xc                  (tensor_tensor ADD, 2x packed)
b0 is read from b on the host (b is constant in this model); if a
non-constant b is ever passed, a general 3-pass variant is built
instead.

w is loaded once as [1, D] (8 KB) and replicated across partitions by
gpsimd SBUF->SBUF broadcast instead of a 1 MB stride-0 HBM broadcast.
The last TAIL_SPLIT tiles are processed in two D/2 chunks so the
drain chain after the final load is roughly halved.
"""

import sys

import numpy as np

sys.path.insert(0, "/opt/trn_rl_repo")

N_CORES = 8
BATCH = 16384
D = 4096
ROWS_PER_CORE = BATCH // N_CORES  # 2048
P = 128
RPP = 1  # rows per partition per tile
BUFS = 5  # 6 overflows SBUF once the tail-chunk pools are added
TAIL_SPLIT = 2  # trailing tiles processed as two D/2 chunks

_NC_CACHE = {}


def _build(rpp=None, bufs=None, io16=True, b0=None, tail_split=None, gp_bcast=True):
    """Build + schedule the single-core SPMD program (same on all cores).

    b0: float -> constant-b fast path (b folded into the ACT bias, b not
    loaded on device). None -> general path (b streamed, 3-pass DVE).
    """
    from contextlib import ExitStack

    import concourse.tile as tile
    from concourse import bacc, mybir

    rpp = RPP if rpp is None else rpp
    bufs = BUFS if bufs is None else bufs
    tail_split = TAIL_SPLIT if tail_split is None else tail_split

    f32 = mybir.dt.float32
    fio = mybir.dt.float16 if io16 else f32
    mult = mybir.AluOpType.mult
    add = mybir.AluOpType.add
    Copy = mybir.ActivationFunctionType.Copy

    nc = bacc.Bacc(
        "TRN2", target_bir_lowering=False, debug=False, num_devices=N_CORES
    )
    x0_d = nc.dram_tensor("x0", [ROWS_PER_CORE, D], fio, kind="ExternalInput").ap()
    xc_d = nc.dram_tensor(
        "x_cross", [ROWS_PER_CORE, D], fio, kind="ExternalInput"
    ).ap()
    w_d = nc.dram_tensor("w", [D], fio, kind="ExternalInput").ap()
    b_d = None
    if b0 is None:
        b_d = nc.dram_tensor("b", [D], fio, kind="ExternalInput").ap()
    out_d = nc.dram_tensor("out", [ROWS_PER_CORE, D], fio, kind="ExternalOutput").ap()

    rows_per_tile = P * rpp
    n_tiles = ROWS_PER_CORE // rows_per_tile
    if rpp != 1:
        tail_split = 0  # chunked tail only implemented for rpp=1
    with tile.TileContext(nc) as tc, ExitStack() as ctx:
        consts = ctx.enter_context(tc.tile_pool(name="consts", bufs=1))
        xc_pool = ctx.enter_context(tc.tile_pool(name="xc", bufs=bufs))
        x0_pool = ctx.enter_context(tc.tile_pool(name="x0", bufs=bufs))
        t_pool = ctx.enter_context(tc.tile_pool(name="t", bufs=2))
        # store staging tiles (ADD output -> store DMA)
        o_pool = ctx.enter_context(tc.tile_pool(name="o", bufs=3))
        # dead stt output; never read, so WAW across tiles only serializes
        # on the in-order DVE stream (keeps the load-side compute decoupled
        # from store drain)
        junk_pool = ctx.enter_context(tc.tile_pool(name="junk", bufs=2))
        s_pool = ctx.enter_context(tc.tile_pool(name="s", bufs=6))
        if tail_split:
            xch_pool = ctx.enter_context(tc.tile_pool(name="xch", bufs=2 * 2))
            x0h_pool = ctx.enter_context(tc.tile_pool(name="x0h", bufs=2 * 2))
            th_pool = ctx.enter_context(tc.tile_pool(name="th", bufs=2))
            oh_pool = ctx.enter_context(tc.tile_pool(name="oh", bufs=2))

        # w replicated across all 128 partitions: one 8 KB HBM load, then
        # an SBUF->SBUF gpsimd broadcast (saves a 1 MB stride-0 HBM read).
        w_t = consts.tile([P, D], fio)
        if gp_bcast:
            w_small = consts.tile([1, D], fio)
            nc.scalar.dma_start(out=w_small[:], in_=w_d[None, :])
            nc.gpsimd.partition_broadcast(w_t[:], w_small[:])
        else:
            nc.scalar.dma_start(out=w_t[:], in_=w_d.partition_broadcast(P))
        if b_d is not None:
            b_t = consts.tile([P, D], fio)
            nc.scalar.dma_start(out=b_t[:], in_=b_d.partition_broadcast(P))

        def load(pool, src, r0, d0, d1):
            t = pool.tile([P, d1 - d0], fio)
            nc.sync.dma_start(
                out=t[:],
                in_=src[r0 : r0 + P, d0:d1].rearrange("(p r) d -> p (r d)", p=P),
            )
            return t

        for i in range(n_tiles):
            r0 = i * rows_per_tile
            chunked = b0 is not None and rpp == 1 and i >= n_tiles - tail_split
            if chunked:
                # two D/2 chunks: halves the drain chain after the last load
                h = D // 2
                xc_a = load(xch_pool, xc_d, r0, 0, h)
                x0_a = load(x0h_pool, x0_d, r0, 0, h)
                xc_b = load(xch_pool, xc_d, r0, h, D)
                x0_b = load(x0h_pool, x0_d, r0, h, D)
                junk_t = junk_pool.tile([P, D], fio)
                sp = s_pool.tile([P, 3], f32)
                for k, xch in ((0, xc_a), (1, xc_b)):
                    nc.vector.scalar_tensor_tensor(
                        out=junk_t[:, k * h : (k + 1) * h],
                        in0=xch[:],
                        scalar=1.0,
                        in1=w_t[:, k * h : (k + 1) * h],
                        op0=mult,
                        op1=mult,
                        accum_out=sp[:, k : k + 1],
                    )
                nc.vector.tensor_add(sp[:, 2:3], sp[:, 0:1], sp[:, 1:2])
                for k, (xch, x0h) in ((0, (xc_a, x0_a)), (1, (xc_b, x0_b))):
                    t_h = th_pool.tile([P, h], fio)
                    nc.scalar.activation(
                        out=t_h[:],
                        in_=x0h[:],
                        func=Copy,
                        bias=float(b0),
                        scale=sp[:, 2:3],
                    )
                    o_h = oh_pool.tile([P, h], fio)
                    nc.vector.tensor_add(o_h[:], t_h[:], xch[:])
                    nc.scalar.dma_start(
                        out=out_d[r0 : r0 + P, k * h : (k + 1) * h].rearrange(
                            "(p r) d -> p (r d)", p=P
                        ),
                        in_=o_h[:],
                    )
                continue

            # [rows_per_tile, D] DRAM block == [P, RPP*D] SBUF tile
            # (partition p holds rows r0 + RPP*p .. r0 + RPP*p + RPP-1)
            xc_t = xc_pool.tile([P, rpp * D], fio)
            nc.sync.dma_start(
                out=xc_t[:],
                in_=xc_d[r0 : r0 + rows_per_tile, :].rearrange(
                    "(p r) d -> p (r d)", p=P
                ),
            )
            x0_t = x0_pool.tile([P, rpp * D], fio)
            nc.sync.dma_start(
                out=x0_t[:],
                in_=x0_d[r0 : r0 + rows_per_tile, :].rearrange(
                    "(p r) d -> p (r d)", p=P
                ),
            )

            o_t = o_pool.tile([P, rpp * D], fio)
            junk_t = junk_pool.tile([P, D], fio)
            s_t = s_pool.tile([P, rpp], f32)
            if b0 is not None:
                t_t = t_pool.tile([P, rpp * D], fio)
            for j in range(rpp):
                ds = slice(j * D, (j + 1) * D)
                # junk = xc * w (dead), s = rowsum(xc * w)
                # (tensor_tensor_reduce's native opcode crashes this runtime;
                # scalar_tensor_tensor's accum_out path does the same thing)
                nc.vector.scalar_tensor_tensor(
                    out=junk_t[:],
                    in0=xc_t[:, ds],
                    scalar=1.0,
                    in1=w_t[:],
                    op0=mult,
                    op1=mult,
                    accum_out=s_t[:, j : j + 1],
                )
                if b0 is not None:
                    # ACT: t = x0 * s + b0 (scale is per-partition)
                    nc.scalar.activation(
                        out=t_t[:, ds],
                        in_=x0_t[:, ds],
                        func=Copy,
                        bias=float(b0),
                        scale=s_t[:, j : j + 1],
                    )
                    # DVE 2x: o = t + xc
                    nc.vector.tensor_add(o_t[:, ds], t_t[:, ds], xc_t[:, ds])
                else:
                    # general-b path: all on DVE
                    t_t = t_pool.tile([P, D], fio, name=f"t{i}_{j}", tag="t")
                    nc.vector.scalar_tensor_tensor(
                        out=t_t[:],
                        in0=x0_t[:, ds],
                        scalar=s_t[:, j : j + 1],
                        in1=xc_t[:, ds],
                        op0=mult,
                        op1=add,
                    )
                    nc.vector.tensor_add(o_t[:, ds], t_t[:], b_t[:])
            # store from the ACT HWDGE ring so loads (SP ring) and stores
            # use separate descriptor generators
            nc.scalar.dma_start(
                out=out_d[r0 : r0 + rows_per_tile, :].rearrange(
                    "(p r) d -> p (r d)", p=P
                ),
                in_=o_t[:],
            )

    nc.compile()
    return nc


def _get_nc(b0, io16=True):
    key = (b0, io16)
    if key not in _NC_CACHE:
        _NC_CACHE[key] = _build(io16=io16, b0=b0)
    return _NC_CACHE[key]


def _run(inputs, trace=False, nc=None, io16=True, **spmd_kwargs):
    """Shard, run on 8 cores, gather. Returns (full_output, BassKernelResults)."""
    from concourse.bass_utils import run_bass_kernel_spmd

    dt = np.float16 if io16 else np.float32
    x0 = np.ascontiguousarray(np.asarray(inputs["x0"]).astype(dt, copy=False))
    xc = np.ascontiguousarray(np.asarray(inputs["x_cross"]).astype(dt, copy=False))
    w = np.ascontiguousarray(np.asarray(inputs["w"]).astype(dt, copy=False))
    b = np.ascontiguousarray(np.asarray(inputs["b"]).astype(dt, copy=False))

    b_np = np.asarray(inputs["b"])
    b0 = float(b_np.reshape(-1)[0]) if np.all(b_np == b_np.reshape(-1)[0]) else None
    if nc is None:
        nc = _get_nc(b0, io16=io16)

    in_maps = []
    for i in range(N_CORES):
        m = {
            "x0": x0[i * ROWS_PER_CORE : (i + 1) * ROWS_PER_CORE],
            "x_cross": xc[i * ROWS_PER_CORE : (i + 1) * ROWS_PER_CORE],
            "w": w,
        }
        if b0 is None:
            m["b"] = b
        in_maps.append(m)

    res = run_bass_kernel_spmd(
        nc, in_maps, core_ids=list(range(N_CORES)), trace=trace, **spmd_kwargs
    )
    out = np.concatenate(
        [res.results[i]["out"].astype(np.float32) for i in range(N_CORES)], axis=0
    )
    return out, res


def kernel(**inputs: np.ndarray) -> np.ndarray:
    out, _ = _run(inputs)
    return out
